# revision 9
# baseline (speedup 1.0000x reference)
"""Trainium2 Bass kernel for nn_CrossAttention (B=8, C=256, CQK=32, H=W=64).

Per-batch cross attention:
    Q = Wq @ xf        [32, 4096]   (+bq)
    K = Wk @ yf        [32, 4096]   (+bk)
    V = Wv @ yf        [256, 4096]  (+bv)
    S = Q^T K          [4096, 4096]
    P = softmax(S, axis=-1)
    out = V @ P^T      [256, 4096]

Sharding: pure data-parallel over batch — core b handles batch b. Weights
replicated. No collectives.

Per-core algorithm (all on-chip, S/P never touch HBM):
  * Q'_rep/K'_rep: projections with the [256,32] transposed weights
    replicated 4x along PE columns, so each 32-partition block of SBUF
    holds a full copy of Q'/K' — feeds 4x row-tiled (K=32) score matmuls.
  * S^T computed in [m, n] layout (m = key index on partitions) via
    4 concurrent row-tiled matmuls (tile_position=(32i,0)), one 512-col
    n-window x 4 m-chunks per group, into 4 PSUM banks.
  * One wide ACT exp over the 4 banks -> P^T tile [128, 2048] in SBUF.
  * out^T[n, c] accumulated in PSUM over all m: stationary = P^T block
    [128m, 128n], moving = V_aug^T[m-chunk] = [V^T | 1] (257 cols). The
    ones column makes PSUM col 256 the softmax denominator for free.
  * normalize by 1/denominator (per-partition broadcast), PE-transpose
    back to [c, n], DMA out.

Unsafe softmax (no max subtraction) is used: scores are ~N(0, 32) for
this problem size, |S| < ~40, exp stays well inside f32 range.

Matmuls use dtype float32r (full f32 storage, single-pass PE matmul —
full rate at free-dim >= 256).
"""

from contextlib import ExitStack

import numpy as np

import concourse.bass as bass
import concourse.mybir as mybir
import concourse.tile as tile
from concourse import bacc
from concourse.bass_utils import run_bass_kernel_spmd
from concourse.masks import make_identity

F32 = mybir.dt.float32
F32R = mybir.dt.float32r
AF = mybir.ActivationFunctionType

B = 8
C = 256          # channels
CQK = 32         # q/k projection dim
HW = 4096        # 64*64 pixels
NW = 8           # n-windows
WIN = HW // NW   # 512 = n-window size
NCH = WIN // 128  # 4 = n-chunks (128) per window
MCH = HW // 128  # 32 m-chunks
MG = MCH // 4    # 8 m-groups of 4 chunks

N_CORES = 8

_CACHE = {}


def _r(ap):
    """View an f32 AP as float32r for full-rate single-pass PE matmul."""
    return ap.bitcast(F32R)


def _build_nc():
    nc = bacc.Bacc("TRN2", target_bir_lowering=False, debug=False)

    x_h = nc.dram_tensor("x", [C, 64, 64], F32, kind="ExternalInput")
    y_h = nc.dram_tensor("y", [C, 64, 64], F32, kind="ExternalInput")
    wq_h = nc.dram_tensor("Wq", [CQK, C], F32, kind="ExternalInput")
    bq_h = nc.dram_tensor("bq", [CQK], F32, kind="ExternalInput")
    wk_h = nc.dram_tensor("Wk", [CQK, C], F32, kind="ExternalInput")
    bk_h = nc.dram_tensor("bk", [CQK], F32, kind="ExternalInput")
    wv_h = nc.dram_tensor("Wv", [C, C], F32, kind="ExternalInput")
    bv_h = nc.dram_tensor("bv", [C], F32, kind="ExternalInput")
    out_h = nc.dram_tensor("out", [C, 64, 64], F32, kind="ExternalOutput")

    x_v = x_h.rearrange("c h w -> c (h w)")
    y_v = y_h.rearrange("c h w -> c (h w)")
    out_v = out_h.rearrange("c h w -> c (h w)")

    with tile.TileContext(nc) as tc, ExitStack() as stk:
        consts = stk.enter_context(tc.tile_pool(name="consts", bufs=1))
        xy = stk.enter_context(tc.tile_pool(name="xy", bufs=1))
        big = stk.enter_context(tc.tile_pool(name="big", bufs=1))
        ppool = stk.enter_context(tc.tile_pool(name="ppool", bufs=3))
        npool = stk.enter_context(tc.tile_pool(name="npool", bufs=6))
        spool = stk.enter_context(tc.tile_pool(name="spool", bufs=4))

        ident = consts.tile([128, 128], F32, name="ident", tag="ident")
        make_identity(nc, ident)

        # ---- load inputs ----
        xin = []
        yin = []
        for cc in range(2):
            xt = xy.tile([128, HW], F32R, name=f"xin{cc}", tag=f"xin{cc}")
            nc.sync.dma_start(
                out=xt, in_=x_v[cc * 128:(cc + 1) * 128, :].bitcast(F32R)
            )
            xin.append(xt)
            yt = xy.tile([128, HW], F32R, name=f"yin{cc}", tag=f"yin{cc}")
            nc.sync.dma_start(
                out=yt, in_=y_v[cc * 128:(cc + 1) * 128, :].bitcast(F32R)
            )
            yin.append(yt)

        wq_sb = consts.tile([CQK, C], F32, name="wq_sb", tag="wq_sb")
        nc.sync.dma_start(out=wq_sb, in_=wq_h[:, :])
        wk_sb = consts.tile([CQK, C], F32, name="wk_sb", tag="wk_sb")
        nc.sync.dma_start(out=wk_sb, in_=wk_h[:, :])
        wv_sb = []
        for cc in range(2):
            t = consts.tile([128, C], F32, name=f"wv_sb{cc}", tag=f"wv_sb{cc}")
            nc.sync.dma_start(out=t, in_=wv_h[cc * 128:(cc + 1) * 128, :])
            wv_sb.append(t)

        # biases: bq/bk replicated 4x partition-wise -> [128, 1]
        bq_rep = consts.tile([128, 1], F32, name="bq_rep", tag="bq_rep")
        bk_rep = consts.tile([128, 1], F32, name="bk_rep", tag="bk_rep")
        for r in range(4):
            nc.sync.dma_start(
                out=bq_rep[32 * r:32 * (r + 1), :],
                in_=bq_h.rearrange("(o u) -> o u", u=1),
            )
            nc.sync.dma_start(
                out=bk_rep[32 * r:32 * (r + 1), :],
                in_=bk_h.rearrange("(o u) -> o u", u=1),
            )
        # bv_aug row: [bv | 1.0], used as the K=1 tail matmul of V_aug^T
        bv_aug = consts.tile([1, C + 2], F32R, name="bv_aug", tag="bv_aug")
        nc.sync.dma_start(
            out=bv_aug[0:1, 0:C], in_=bv_h.rearrange("(u c) -> u c", u=1).bitcast(F32R)
        )
        nc.vector.memset(bv_aug[0:1, C:C + 1], 1.0)
        nc.vector.memset(bv_aug[0:1, C + 1:C + 2], 0.0)
        ones_row = consts.tile([1, 128], F32R, name="ones_row", tag="ones_row")
        nc.vector.memset(ones_row, 1.0)

        # persistent big tensors
        qrep = big.tile([128, HW], F32R, name="qrep", tag="qrep")
        krep = big.tile([128, HW], F32R, name="krep", tag="krep")
        vaug = big.tile([128, MCH, C + 2], F32R, name="vaug", tag="vaug")

        # stationaries for projections
        wqT_rep = []
        wkT_rep = []
        wvT_aug = []
        for cc in range(2):
            wqT_rep.append(
                consts.tile([128, 128], F32R, name=f"wqT{cc}", tag=f"wqT{cc}")
            )
            wkT_rep.append(
                consts.tile([128, 128], F32R, name=f"wkT{cc}", tag=f"wkT{cc}")
            )
            t = consts.tile([128, C + 2], F32R, name=f"wvT{cc}", tag=f"wvT{cc}")
            nc.vector.memset(t[:, C:C + 2], 0.0)
            wvT_aug.append(t)

        # ---- prep-phase PSUM pools (closed before the main loop) ----
        with tc.tile_pool(name="psum_pp", bufs=4, space="PSUM") as psum_pp, \
             tc.tile_pool(name="psum_tp", bufs=2, space="PSUM") as psum_tp:
            # transpose Wq/Wk chunks -> replicated stationaries
            for cc in range(2):
                tq = psum_tp.tile([128, 128], F32, name=f"tq{cc}", tag="tp")
                nc.tensor.transpose(
                    tq[:, 0:CQK],
                    wq_sb[0:CQK, cc * 128:(cc + 1) * 128],
                    ident[0:CQK, 0:CQK],
                )
                for r in range(4):
                    nc.vector.tensor_copy(
                        out=wqT_rep[cc][:, 32 * r:32 * (r + 1)], in_=tq[:, 0:CQK]
                    )
                tk = psum_tp.tile([128, 128], F32, name=f"tk{cc}", tag="tp")
                nc.tensor.transpose(
                    tk[:, 0:CQK],
                    wk_sb[0:CQK, cc * 128:(cc + 1) * 128],
                    ident[0:CQK, 0:CQK],
                )
                for r in range(4):
                    nc.vector.tensor_copy(
                        out=wkT_rep[cc][:, 32 * r:32 * (r + 1)], in_=tk[:, 0:CQK]
                    )
            # transpose Wv -> wvT_aug ([c' part, c free], col 256 = 0)
            for ccp in range(2):
                for cc in range(2):
                    tv = psum_tp.tile([128, 128], F32, name=f"tv{ccp}{cc}", tag="tp")
                    nc.tensor.transpose(
                        tv,
                        wv_sb[cc][:, ccp * 128:(ccp + 1) * 128],
                        ident,
                    )
                    nc.vector.tensor_copy(
                        out=wvT_aug[ccp][:, cc * 128:(cc + 1) * 128], in_=tv
                    )

            # Q'_rep / K'_rep projections (out = 4 stacked replicas of [32, n])
            for nn in range(NW):
                ns = slice(nn * WIN, (nn + 1) * WIN)
                qp = psum_pp.tile([128, WIN], F32, name=f"qp{nn}", tag="pp")
                nc.tensor.matmul(
                    out=qp, lhsT=wqT_rep[0], rhs=xin[0][:, ns],
                    start=True, stop=False,
                )
                nc.tensor.matmul(
                    out=qp, lhsT=wqT_rep[1], rhs=xin[1][:, ns],
                    start=False, stop=True,
                )
                nc.vector.tensor_scalar_add(out=qrep[:, ns], in0=qp, scalar1=bq_rep)
                kp = psum_pp.tile([128, WIN], F32, name=f"kp{nn}", tag="pp")
                nc.tensor.matmul(
                    out=kp, lhsT=wkT_rep[0], rhs=yin[0][:, ns],
                    start=True, stop=False,
                )
                nc.tensor.matmul(
                    out=kp, lhsT=wkT_rep[1], rhs=yin[1][:, ns],
                    start=False, stop=True,
                )
                nc.vector.tensor_scalar_add(out=krep[:, ns], in0=kp, scalar1=bk_rep)

            # V_aug^T: per m-chunk [128, 257] = yf^T Wv^T + [bv | 1]
            for mc in range(MCH):
                ms = slice(mc * 128, (mc + 1) * 128)
                vp = psum_pp.tile([128, WIN], F32, name=f"vp{mc}", tag="pp")
                nc.tensor.matmul(
                    out=vp[:, 0:C + 2], lhsT=yin[0][:, ms], rhs=wvT_aug[0],
                    start=True, stop=False,
                )
                nc.tensor.matmul(
                    out=vp[:, 0:C + 2], lhsT=yin[1][:, ms], rhs=wvT_aug[1],
                    start=False, stop=False,
                )
                nc.tensor.matmul(
                    out=vp[:, 0:C + 2], lhsT=ones_row, rhs=bv_aug,
                    start=False, stop=True,
                )
                nc.vector.tensor_copy(out=vaug[:, mc, :], in_=vp[:, 0:C + 2])

        # ---- main loop ----
        with tc.tile_pool(name="psum_s", bufs=1, space="PSUM") as psum_s, \
             tc.tile_pool(name="psum_o", bufs=4, space="PSUM") as psum_o:

            def emit_s_group(w, g):
                """4 concurrent row-tiled score matmuls: S^T[m-chunks 4g..4g+3,
                n-window w] into 4 PSUM banks of one [128, 2048] tile."""
                sp = psum_s.tile([128, 4 * WIN], F32, name=f"sp{w}_{g}", tag="s")
                for i in range(4):
                    mc = 4 * g + i
                    prt = slice(32 * i, 32 * (i + 1))
                    nc.tensor.matmul(
                        out=sp[:, i * WIN:(i + 1) * WIN],
                        lhsT=krep[prt, mc * 128:(mc + 1) * 128],
                        rhs=qrep[prt, w * WIN:(w + 1) * WIN],
                        start=True, stop=True,
                        tile_position=(32 * i, 0),
                    )
                return sp

            for w in range(NW):
                opsum = [
                    psum_o.tile([128, C + 2], F32, name=f"o{w}_{j}", tag="o")
                    for j in range(NCH)
                ]
                sp = emit_s_group(w, 0)
                for g in range(MG):
                    pt = ppool.tile([128, 4 * WIN], F32R, name=f"pt{w}_{g}", tag="pt")
                    nc.scalar.activation(out=pt, in_=sp, func=AF.Exp)
                    if g + 1 < MG:
                        sp = emit_s_group(w, g + 1)
                    for i in range(4):
                        mc = 4 * g + i
                        for j in range(NCH):
                            nc.tensor.matmul(
                                out=opsum[j][:, 0:C + 2],
                                lhsT=pt[:, i * WIN + j * 128:i * WIN + (j + 1) * 128],
                                rhs=vaug[:, mc, :],
                                start=(mc == 0), stop=(mc == MCH - 1),
                            )
                # normalize: out^T[n, c] * (1/denom[n]); denom is col 256
                nsbs = []
                for j in range(NCH):
                    rec = npool.tile([128, 1], F32, name=f"rec{w}_{j}", tag="rec")
                    nc.vector.reciprocal(out=rec, in_=opsum[j][:, C:C + 1])
                    nsb = npool.tile([128, C], F32, name=f"nsb{w}_{j}", tag="nsb")
                    nc.vector.tensor_scalar_mul(
                        out=nsb, in0=opsum[j][:, 0:C], scalar1=rec
                    )
                    nsbs.append(nsb)
                # transpose back to [c, n] and store
                ost = [
                    spool.tile([128, WIN], F32, name=f"ost{w}_{cc}", tag="ost")
                    for cc in range(2)
                ]
                for j in range(NCH):
                    for cc in range(2):
                        tp = psum_o.tile([128, C + 2], F32, name=f"tp{w}_{j}{cc}", tag="o")
                        nc.tensor.transpose(
                            tp[:, 0:128],
                            nsbs[j][:, cc * 128:(cc + 1) * 128],
                            ident,
                        )
                        nc.vector.tensor_copy(
                            out=ost[cc][:, j * 128:(j + 1) * 128], in_=tp[:, 0:128]
                        )
                for cc in range(2):
                    nc.sync.dma_start(
                        out=out_v[cc * 128:(cc + 1) * 128, w * WIN:(w + 1) * WIN],
                        in_=ost[cc],
                    )

    nc.compile()
    return nc


def _get_nc():
    if "nc" not in _CACHE:
        _CACHE["nc"] = _build_nc()
    return _CACHE["nc"]


class _Runner:
    """One-time jitted SPMD executor for the bass program (mirrors
    bass2jax.run_bass_via_pjrt, but keeps the jitted callable for reuse)."""

    def __init__(self, nc):
        import jax
        import concourse.mybir as mybir_
        from concourse import bass2jax
        from jax.experimental.shard_map import shard_map
        from jax.sharding import Mesh, PartitionSpec

        bass2jax.install_neuronx_cc_hook()
        self.jax = jax
        self.nc = nc

        partition_name = (
            nc.partition_id_tensor.name if nc.partition_id_tensor else None
        )
        in_names, out_names, out_avals, zero_outs = [], [], [], []
        for alloc in nc.m.functions[0].allocations:
            if not isinstance(alloc, mybir_.MemoryLocationSet):
                continue
            name = alloc.memorylocations[0].name
            if alloc.kind == "ExternalInput":
                if name != partition_name:
                    in_names.append(name)
            elif alloc.kind == "ExternalOutput":
                out_names.append(name)
                shape = tuple(alloc.tensor_shape)
                dtype = mybir_.dt.np(alloc.dtype)
                out_avals.append(jax.core.ShapedArray(shape, dtype))
                zero_outs.append(np.zeros(shape, dtype))
        self.in_names = list(in_names)
        self.out_names = out_names
        self.zero_outs = zero_outs
        n_params = len(in_names)
        n_outs = len(out_avals)
        all_in_names = in_names + out_names
        if partition_name is not None:
            all_in_names = all_in_names + [partition_name]
        donate = tuple(range(n_params, n_params + n_outs))
        self.n_params = n_params

        def _body(*args):
            operands = list(args)
            if partition_name is not None:
                operands.append(bass2jax.partition_id_tensor())
            outs = bass2jax._bass_exec_p.bind(
                *operands,
                out_avals=tuple(out_avals),
                in_names=tuple(all_in_names),
                out_names=tuple(out_names),
                lowering_input_output_aliases=(),
                sim_require_finite=True,
                sim_require_nnan=True,
                nc=nc,
            )
            return tuple(outs)

        devices = jax.devices()[:N_CORES]
        self.mesh = Mesh(np.asarray(devices), ("core",))
        in_specs = (PartitionSpec("core"),) * (n_params + n_outs)
        out_specs = (PartitionSpec("core"),) * n_outs
        self.sharded = jax.jit(
            shard_map(
                _body, mesh=self.mesh, in_specs=in_specs, out_specs=out_specs,
                check_rep=False,
            ),
            donate_argnums=donate,
            keep_unused=True,
        )

    def make_zeros(self):
        return [
            np.zeros((N_CORES * z.shape[0], *z.shape[1:]), z.dtype)
            for z in self.zero_outs
        ]

    def concat_inputs(self, in_maps):
        return [
            np.concatenate([np.asarray(m[name]) for m in in_maps], axis=0)
            for name in self.in_names
        ]

    def run(self, concat_in, zeros):
        outs = self.sharded(*concat_in, *zeros)
        return outs


def _get_runner():
    if "runner" not in _CACHE:
        _CACHE["runner"] = _Runner(_get_nc())
    return _CACHE["runner"]


def kernel(x, y, Wq, bq, Wk, bk, Wv, bv):
    r = _get_runner()
    x = np.ascontiguousarray(np.asarray(x, dtype=np.float32))
    y = np.ascontiguousarray(np.asarray(y, dtype=np.float32))
    Wq = np.ascontiguousarray(np.asarray(Wq, dtype=np.float32))
    bq = np.ascontiguousarray(np.asarray(bq, dtype=np.float32))
    Wk = np.ascontiguousarray(np.asarray(Wk, dtype=np.float32))
    bk = np.ascontiguousarray(np.asarray(bk, dtype=np.float32))
    Wv = np.ascontiguousarray(np.asarray(Wv, dtype=np.float32))
    bv = np.ascontiguousarray(np.asarray(bv, dtype=np.float32))

    in_maps = [
        {
            "x": x[b], "y": y[b],
            "Wq": Wq, "bq": bq, "Wk": Wk, "bk": bk, "Wv": Wv, "bv": bv,
        }
        for b in range(B)
    ]
    concat_in = r.concat_inputs(in_maps)
    outs = r.run(concat_in, r.make_zeros())
    out = np.asarray(outs[0])  # [8*256, 64, 64]
    return out.reshape(B, C, 64, 64)


# revision 12
# speedup vs baseline: 11.7456x; 11.7456x over previous
"""Trainium2 Bass kernel for nn_CrossAttention (B=8, C=256, CQK=32, H=W=64).

Per-batch cross attention:
    Q = Wq @ xf        [32, 4096]   (+bq)
    K = Wk @ yf        [32, 4096]   (+bk)
    V = Wv @ yf        [256, 4096]  (+bv)
    S = Q^T K          [4096, 4096]
    P = softmax(S, axis=-1)
    out = V @ P^T      [256, 4096]

Sharding: pure data-parallel over batch — core b handles batch b. Weights
replicated. No collectives.

Per-core algorithm (all on-chip, S/P never touch HBM):
  * Q'_rep/K'_rep: projections with the [256,32] transposed weights
    replicated 4x along PE columns, so each 32-partition block of SBUF
    holds a full copy of Q'/K' — feeds 4x row-tiled (K=32) score matmuls.
  * S^T computed in [m, n] layout (m = key index on partitions) via
    4 concurrent row-tiled matmuls (tile_position=(32i,0)), one 512-col
    n-window x 4 m-chunks per group, into 4 PSUM banks.
  * One wide ACT exp over the 4 banks -> P^T tile [128, 2048] in SBUF.
  * out^T[n, c] accumulated in PSUM over all m: stationary = P^T block
    [128m, 128n], moving = V_aug^T[m-chunk] = [V^T | 1] (257 cols). The
    ones column makes PSUM col 256 the softmax denominator for free.
  * normalize by 1/denominator (per-partition broadcast), PE-transpose
    back to [c, n], DMA out.

Unsafe softmax (no max subtraction) is used: scores are ~N(0, 32) for
this problem size, |S| < ~40, exp stays well inside f32 range.

Matmuls use dtype float32r (full f32 storage, single-pass PE matmul —
full rate at free-dim >= 256).
"""

from contextlib import ExitStack

import numpy as np

import concourse.bass as bass
import concourse.mybir as mybir
import concourse.tile as tile
from concourse import bacc
from concourse.bass_utils import run_bass_kernel_spmd
from concourse.masks import make_identity

F32 = mybir.dt.float32
F32R = mybir.dt.float32r
BF16 = mybir.dt.bfloat16
AF = mybir.ActivationFunctionType

# dtype for the big out^T = P^T-stationary x V_aug matmuls (1024 of them).
# float32r: ~10-bit-mantissa single-pass fp32; bfloat16: guaranteed
# 1 cyc/row + fast weight load.
OUT_BF16 = False

B = 8
C = 256          # channels
CQK = 32         # q/k projection dim
HW = 4096        # 64*64 pixels
NW = 8           # n-windows
WIN = HW // NW   # 512 = n-window size
NCH = WIN // 128  # 4 = n-chunks (128) per window
MCH = HW // 128  # 32 m-chunks
MG = MCH // 4    # 8 m-groups of 4 chunks

N_CORES = 8

_CACHE = {}


def _r(ap):
    """View an f32 AP as float32r for full-rate single-pass PE matmul."""
    return ap.bitcast(F32R)


def _build_nc(reps=1):
    nc = bacc.Bacc("TRN2", target_bir_lowering=False, debug=False)

    x_h = nc.dram_tensor("x", [C, 64, 64], F32, kind="ExternalInput")
    y_h = nc.dram_tensor("y", [C, 64, 64], F32, kind="ExternalInput")
    wq_h = nc.dram_tensor("Wq", [CQK, C], F32, kind="ExternalInput")
    bq_h = nc.dram_tensor("bq", [CQK], F32, kind="ExternalInput")
    wk_h = nc.dram_tensor("Wk", [CQK, C], F32, kind="ExternalInput")
    bk_h = nc.dram_tensor("bk", [CQK], F32, kind="ExternalInput")
    wv_h = nc.dram_tensor("Wv", [C, C], F32, kind="ExternalInput")
    bv_h = nc.dram_tensor("bv", [C], F32, kind="ExternalInput")
    out_h = nc.dram_tensor("out", [C, 64, 64], F32, kind="ExternalOutput")

    x_v = x_h.rearrange("c h w -> c (h w)")
    y_v = y_h.rearrange("c h w -> c (h w)")
    out_v = out_h.rearrange("c h w -> c (h w)")

    def emit_once(tc, nc, rep):
      with ExitStack() as stk:
        consts = stk.enter_context(tc.tile_pool(name=f"consts{rep}", bufs=1))
        xy = stk.enter_context(tc.tile_pool(name=f"xy{rep}", bufs=1))
        big = stk.enter_context(tc.tile_pool(name=f"big{rep}", bufs=1))
        ppool = stk.enter_context(tc.tile_pool(name=f"ppool{rep}", bufs=3))
        npool = stk.enter_context(tc.tile_pool(name=f"npool{rep}", bufs=6))
        spool = stk.enter_context(tc.tile_pool(name=f"spool{rep}", bufs=4))

        ident = consts.tile([128, 128], F32, name="ident", tag="ident")
        make_identity(nc, ident)

        # ---- load inputs ----
        xin = []
        yin = []
        for cc in range(2):
            xt = xy.tile([128, HW], F32R, name=f"xin{cc}", tag=f"xin{cc}")
            nc.sync.dma_start(
                out=xt, in_=x_v[cc * 128:(cc + 1) * 128, :].bitcast(F32R)
            )
            xin.append(xt)
            yt = xy.tile([128, HW], F32R, name=f"yin{cc}", tag=f"yin{cc}")
            nc.sync.dma_start(
                out=yt, in_=y_v[cc * 128:(cc + 1) * 128, :].bitcast(F32R)
            )
            yin.append(yt)

        wq_sb = consts.tile([CQK, C], F32, name="wq_sb", tag="wq_sb")
        nc.sync.dma_start(out=wq_sb, in_=wq_h[:, :])
        wk_sb = consts.tile([CQK, C], F32, name="wk_sb", tag="wk_sb")
        nc.sync.dma_start(out=wk_sb, in_=wk_h[:, :])
        wv_sb = []
        for cc in range(2):
            t = consts.tile([128, C], F32, name=f"wv_sb{cc}", tag=f"wv_sb{cc}")
            nc.sync.dma_start(out=t, in_=wv_h[cc * 128:(cc + 1) * 128, :])
            wv_sb.append(t)

        # biases: bq/bk replicated 4x partition-wise -> [128, 1]
        bq_rep = consts.tile([128, 1], F32, name="bq_rep", tag="bq_rep")
        bk_rep = consts.tile([128, 1], F32, name="bk_rep", tag="bk_rep")
        for r in range(4):
            nc.sync.dma_start(
                out=bq_rep[32 * r:32 * (r + 1), :],
                in_=bq_h.rearrange("(o u) -> o u", u=1),
            )
            nc.sync.dma_start(
                out=bk_rep[32 * r:32 * (r + 1), :],
                in_=bk_h.rearrange("(o u) -> o u", u=1),
            )
        # bv_aug row: [bv | 1.0], used as the K=1 tail matmul of V_aug^T
        bv_aug = consts.tile([1, C + 2], F32R, name="bv_aug", tag="bv_aug")
        nc.sync.dma_start(
            out=bv_aug[0:1, 0:C], in_=bv_h.rearrange("(u c) -> u c", u=1).bitcast(F32R)
        )
        # f32r memsets are rejected by walrus; memset f32 scratch, then
        # DVE-copy (which rounds) into the f32r tiles.
        scr = consts.tile([128, 130], F32, name="scr", tag="scr")
        nc.vector.memset(scr[:, 0:2], 0.0)
        nc.vector.memset(scr[:, 2:130], 1.0)
        nc.vector.tensor_copy(out=bv_aug[0:1, C:C + 1], in_=scr[0:1, 2:3])
        nc.vector.tensor_copy(out=bv_aug[0:1, C + 1:C + 2], in_=scr[0:1, 0:1])
        ones_row = consts.tile([1, 128], F32R, name="ones_row", tag="ones_row")
        nc.vector.tensor_copy(out=ones_row, in_=scr[0:1, 2:130])

        # persistent big tensors
        qrep = big.tile([128, HW], F32R, name="qrep", tag="qrep")
        krep = big.tile([128, HW], F32R, name="krep", tag="krep")
        vaug = big.tile([128, MCH, C + 2], BF16 if OUT_BF16 else F32R, name="vaug", tag="vaug")

        # stationaries for projections
        wqT_rep = []
        wkT_rep = []
        wvT_aug = []
        for cc in range(2):
            wqT_rep.append(
                consts.tile([128, 128], F32R, name=f"wqT{cc}", tag=f"wqT{cc}")
            )
            wkT_rep.append(
                consts.tile([128, 128], F32R, name=f"wkT{cc}", tag=f"wkT{cc}")
            )
            t = consts.tile([128, C + 2], F32R, name=f"wvT{cc}", tag=f"wvT{cc}")
            nc.vector.tensor_copy(out=t[:, C:C + 2], in_=scr[:, 0:2])
            wvT_aug.append(t)

        # ---- prep-phase PSUM pools (closed before the main loop) ----
        with tc.tile_pool(name="psum_pp", bufs=4, space="PSUM") as psum_pp, \
             tc.tile_pool(name="psum_tp", bufs=2, space="PSUM") as psum_tp:
            # transpose Wq/Wk chunks -> replicated stationaries
            for cc in range(2):
                tq = psum_tp.tile([128, 128], F32, name=f"tq{cc}", tag="tp")
                nc.tensor.transpose(
                    tq[:, 0:CQK],
                    wq_sb[0:CQK, cc * 128:(cc + 1) * 128],
                    ident[0:CQK, 0:CQK],
                )
                for r in range(4):
                    nc.vector.tensor_copy(
                        out=wqT_rep[cc][:, 32 * r:32 * (r + 1)], in_=tq[:, 0:CQK]
                    )
                tk = psum_tp.tile([128, 128], F32, name=f"tk{cc}", tag="tp")
                nc.tensor.transpose(
                    tk[:, 0:CQK],
                    wk_sb[0:CQK, cc * 128:(cc + 1) * 128],
                    ident[0:CQK, 0:CQK],
                )
                for r in range(4):
                    nc.vector.tensor_copy(
                        out=wkT_rep[cc][:, 32 * r:32 * (r + 1)], in_=tk[:, 0:CQK]
                    )
            # transpose Wv -> wvT_aug ([c' part, c free], col 256 = 0)
            for ccp in range(2):
                for cc in range(2):
                    tv = psum_tp.tile([128, 128], F32, name=f"tv{ccp}{cc}", tag="tp")
                    nc.tensor.transpose(
                        tv,
                        wv_sb[cc][:, ccp * 128:(ccp + 1) * 128],
                        ident,
                    )
                    nc.vector.tensor_copy(
                        out=wvT_aug[ccp][:, cc * 128:(cc + 1) * 128], in_=tv
                    )

            # Q'_rep / K'_rep projections (out = 4 stacked replicas of [32, n])
            for nn in range(NW):
                ns = slice(nn * WIN, (nn + 1) * WIN)
                qp = psum_pp.tile([128, WIN], F32, name=f"qp{nn}", tag="pp")
                nc.tensor.matmul(
                    out=qp, lhsT=wqT_rep[0], rhs=xin[0][:, ns],
                    start=True, stop=False,
                )
                nc.tensor.matmul(
                    out=qp, lhsT=wqT_rep[1], rhs=xin[1][:, ns],
                    start=False, stop=True,
                )
                nc.vector.tensor_scalar_add(out=qrep[:, ns], in0=qp, scalar1=bq_rep)
                kp = psum_pp.tile([128, WIN], F32, name=f"kp{nn}", tag="pp")
                nc.tensor.matmul(
                    out=kp, lhsT=wkT_rep[0], rhs=yin[0][:, ns],
                    start=True, stop=False,
                )
                nc.tensor.matmul(
                    out=kp, lhsT=wkT_rep[1], rhs=yin[1][:, ns],
                    start=False, stop=True,
                )
                nc.vector.tensor_scalar_add(out=krep[:, ns], in0=kp, scalar1=bk_rep)

            # V_aug^T: per m-chunk [128, 257] = yf^T Wv^T + [bv | 1]
            for mc in range(MCH):
                ms = slice(mc * 128, (mc + 1) * 128)
                vp = psum_pp.tile([128, WIN], F32, name=f"vp{mc}", tag="pp")
                nc.tensor.matmul(
                    out=vp[:, 0:C + 2], lhsT=yin[0][:, ms], rhs=wvT_aug[0],
                    start=True, stop=False,
                )
                nc.tensor.matmul(
                    out=vp[:, 0:C + 2], lhsT=yin[1][:, ms], rhs=wvT_aug[1],
                    start=False, stop=False,
                )
                nc.tensor.matmul(
                    out=vp[:, 0:C + 2], lhsT=ones_row, rhs=bv_aug,
                    start=False, stop=True,
                )
                nc.vector.tensor_copy(out=vaug[:, mc, :], in_=vp[:, 0:C + 2])

        # ---- main loop ----
        with tc.tile_pool(name="psum_s", bufs=1, space="PSUM") as psum_s, \
             tc.tile_pool(name="psum_o", bufs=4, space="PSUM") as psum_o:

            def emit_s_group(w, g):
                """4 concurrent row-tiled score matmuls: S^T[m-chunks 4g..4g+3,
                n-window w] into 4 PSUM banks of one [128, 2048] tile."""
                sp = psum_s.tile([128, 4 * WIN], F32, name=f"sp{w}_{g}", tag="s")
                for i in range(4):
                    mc = 4 * g + i
                    prt = slice(32 * i, 32 * (i + 1))
                    nc.tensor.matmul(
                        out=sp[:, i * WIN:(i + 1) * WIN],
                        lhsT=krep[prt, mc * 128:(mc + 1) * 128],
                        rhs=qrep[prt, w * WIN:(w + 1) * WIN],
                        start=True, stop=True,
                        tile_position=(32 * i, 0),
                    )
                return sp

            for w in range(NW):
                opsum = [
                    psum_o.tile([128, C + 2], F32, name=f"o{w}_{j}", tag="o")
                    for j in range(NCH)
                ]
                sp = emit_s_group(w, 0)
                for g in range(MG):
                    pt = ppool.tile([128, 4 * WIN], BF16 if OUT_BF16 else F32R, name=f"pt{w}_{g}", tag="pt")
                    nc.scalar.activation(out=pt, in_=sp, func=AF.Exp)
                    if g + 1 < MG:
                        sp = emit_s_group(w, g + 1)
                    for i in range(4):
                        mc = 4 * g + i
                        for j in range(NCH):
                            nc.tensor.matmul(
                                out=opsum[j][:, 0:C + 2],
                                lhsT=pt[:, i * WIN + j * 128:i * WIN + (j + 1) * 128],
                                rhs=vaug[:, mc, :],
                                start=(mc == 0), stop=(mc == MCH - 1),
                            )
                # normalize: out^T[n, c] * (1/denom[n]); denom is col 256
                nsbs = []
                for j in range(NCH):
                    rec = npool.tile([128, 1], F32, name=f"rec{w}_{j}", tag="rec")
                    nc.vector.reciprocal(out=rec, in_=opsum[j][:, C:C + 1])
                    nsb = npool.tile([128, C], F32, name=f"nsb{w}_{j}", tag="nsb")
                    nc.vector.tensor_scalar_mul(
                        out=nsb, in0=opsum[j][:, 0:C], scalar1=rec
                    )
                    nsbs.append(nsb)
                # transpose back to [c, n] and store
                ost = [
                    spool.tile([128, WIN], F32, name=f"ost{w}_{cc}", tag="ost")
                    for cc in range(2)
                ]
                for j in range(NCH):
                    for cc in range(2):
                        tp = psum_o.tile([128, C + 2], F32, name=f"tp{w}_{j}{cc}", tag="o")
                        nc.tensor.transpose(
                            tp[:, 0:128],
                            nsbs[j][:, cc * 128:(cc + 1) * 128],
                            ident,
                        )
                        nc.vector.tensor_copy(
                            out=ost[cc][:, j * 128:(j + 1) * 128], in_=tp[:, 0:128]
                        )
                for cc in range(2):
                    nc.sync.dma_start(
                        out=out_v[cc * 128:(cc + 1) * 128, w * WIN:(w + 1) * WIN],
                        in_=ost[cc],
                    )

    with tile.TileContext(nc) as tc:
        for rep in range(reps):
            emit_once(tc, nc, rep)

    nc.compile()
    return nc


def _get_nc():
    if "nc" not in _CACHE:
        _CACHE["nc"] = _build_nc()
    return _CACHE["nc"]


class _Runner:
    """One-time jitted SPMD executor for the bass program (mirrors
    bass2jax.run_bass_via_pjrt, but keeps the jitted callable for reuse)."""

    def __init__(self, nc):
        import jax
        import concourse.mybir as mybir_
        from concourse import bass2jax
        from jax.experimental.shard_map import shard_map
        from jax.sharding import Mesh, PartitionSpec

        bass2jax.install_neuronx_cc_hook()
        self.jax = jax
        self.nc = nc

        partition_name = (
            nc.partition_id_tensor.name if nc.partition_id_tensor else None
        )
        in_names, out_names, out_avals, zero_outs = [], [], [], []
        for alloc in nc.m.functions[0].allocations:
            if not isinstance(alloc, mybir_.MemoryLocationSet):
                continue
            name = alloc.memorylocations[0].name
            if alloc.kind == "ExternalInput":
                if name != partition_name:
                    in_names.append(name)
            elif alloc.kind == "ExternalOutput":
                out_names.append(name)
                shape = tuple(alloc.tensor_shape)
                dtype = mybir_.dt.np(alloc.dtype)
                out_avals.append(jax.core.ShapedArray(shape, dtype))
                zero_outs.append(np.zeros(shape, dtype))
        self.in_names = list(in_names)
        self.out_names = out_names
        self.zero_outs = zero_outs
        n_params = len(in_names)
        n_outs = len(out_avals)
        all_in_names = in_names + out_names
        if partition_name is not None:
            all_in_names = all_in_names + [partition_name]
        donate = tuple(range(n_params, n_params + n_outs))
        self.n_params = n_params

        def _body(*args):
            operands = list(args)
            if partition_name is not None:
                operands.append(bass2jax.partition_id_tensor())
            outs = bass2jax._bass_exec_p.bind(
                *operands,
                out_avals=tuple(out_avals),
                in_names=tuple(all_in_names),
                out_names=tuple(out_names),
                lowering_input_output_aliases=(),
                sim_require_finite=True,
                sim_require_nnan=True,
                nc=nc,
            )
            return tuple(outs)

        devices = jax.devices()[:N_CORES]
        self.mesh = Mesh(np.asarray(devices), ("core",))
        in_specs = (PartitionSpec("core"),) * (n_params + n_outs)
        out_specs = (PartitionSpec("core"),) * n_outs
        self.sharded = jax.jit(
            shard_map(
                _body, mesh=self.mesh, in_specs=in_specs, out_specs=out_specs,
                check_rep=False,
            ),
            donate_argnums=donate,
            keep_unused=True,
        )

    def make_zeros(self):
        return [
            np.zeros((N_CORES * z.shape[0], *z.shape[1:]), z.dtype)
            for z in self.zero_outs
        ]

    def concat_inputs(self, in_maps):
        return [
            np.concatenate([np.asarray(m[name]) for m in in_maps], axis=0)
            for name in self.in_names
        ]

    def run(self, concat_in, zeros):
        outs = self.sharded(*concat_in, *zeros)
        return outs


def _get_runner():
    if "runner" not in _CACHE:
        _CACHE["runner"] = _Runner(_get_nc())
    return _CACHE["runner"]


def kernel(x, y, Wq, bq, Wk, bk, Wv, bv):
    r = _get_runner()
    x = np.ascontiguousarray(np.asarray(x, dtype=np.float32))
    y = np.ascontiguousarray(np.asarray(y, dtype=np.float32))
    Wq = np.ascontiguousarray(np.asarray(Wq, dtype=np.float32))
    bq = np.ascontiguousarray(np.asarray(bq, dtype=np.float32))
    Wk = np.ascontiguousarray(np.asarray(Wk, dtype=np.float32))
    bk = np.ascontiguousarray(np.asarray(bk, dtype=np.float32))
    Wv = np.ascontiguousarray(np.asarray(Wv, dtype=np.float32))
    bv = np.ascontiguousarray(np.asarray(bv, dtype=np.float32))

    in_maps = [
        {
            "x": x[b], "y": y[b],
            "Wq": Wq, "bq": bq, "Wk": Wk, "bk": bk, "Wv": Wv, "bv": bv,
        }
        for b in range(B)
    ]
    concat_in = r.concat_inputs(in_maps)
    outs = r.run(concat_in, r.make_zeros())
    out = np.asarray(outs[0])  # [8*256, 64, 64]
    return out.reshape(B, C, 64, 64)


# revision 14
# speedup vs baseline: 13.2881x; 1.1313x over previous
"""Trainium2 Bass kernel for nn_CrossAttention (B=8, C=256, CQK=32, H=W=64).

Per-batch cross attention:
    Q = Wq @ xf        [32, 4096]   (+bq)
    K = Wk @ yf        [32, 4096]   (+bk)
    V = Wv @ yf        [256, 4096]  (+bv)
    S = Q^T K          [4096, 4096]
    P = softmax(S, axis=-1)
    out = V @ P^T      [256, 4096]

Sharding: pure data-parallel over batch — core b handles batch b. Weights
replicated. No collectives.

Per-core algorithm (all on-chip, S/P never touch HBM):
  * Q'_rep/K'_rep: projections with the [256,32] transposed weights
    replicated 4x along PE columns, so each 32-partition block of SBUF
    holds a full copy of Q'/K' — feeds 4x row-tiled (K=32) score matmuls.
  * S^T computed in [m, n] layout (m = key index on partitions) via
    4 concurrent row-tiled matmuls (tile_position=(32i,0)), one 512-col
    n-window x 4 m-chunks per group, into 4 PSUM banks.
  * One wide ACT exp over the 4 banks -> P^T tile [128, 2048] in SBUF.
  * out^T[n, c] accumulated in PSUM over all m: stationary = P^T block
    [128m, 128n], moving = V_aug^T[m-chunk] = [V^T | 1] (257 cols). The
    ones column makes PSUM col 256 the softmax denominator for free.
  * normalize by 1/denominator (per-partition broadcast), PE-transpose
    back to [c, n], DMA out.

Unsafe softmax (no max subtraction) is used: scores are ~N(0, 32) for
this problem size, |S| < ~40, exp stays well inside f32 range.

Matmuls use dtype float32r (full f32 storage, single-pass PE matmul —
full rate at free-dim >= 256).
"""

from contextlib import ExitStack

import numpy as np

import concourse.bass as bass
import concourse.mybir as mybir
import concourse.tile as tile
from concourse import bacc
from concourse.bass_utils import run_bass_kernel_spmd
from concourse.masks import make_identity

F32 = mybir.dt.float32
F32R = mybir.dt.float32r
BF16 = mybir.dt.bfloat16
AF = mybir.ActivationFunctionType

# dtype for the big out^T = P^T-stationary x V_aug matmuls (1024 of them).
# float32r: ~10-bit-mantissa single-pass fp32; bfloat16: guaranteed
# 1 cyc/row + fast weight load.
OUT_BF16 = True

B = 8
C = 256          # channels
CQK = 32         # q/k projection dim
HW = 4096        # 64*64 pixels
NW = 8           # n-windows
WIN = HW // NW   # 512 = n-window size
NCH = WIN // 128  # 4 = n-chunks (128) per window
MCH = HW // 128  # 32 m-chunks
MG = MCH // 4    # 8 m-groups of 4 chunks

N_CORES = 8

_CACHE = {}


def _r(ap):
    """View an f32 AP as float32r for full-rate single-pass PE matmul."""
    return ap.bitcast(F32R)


def _build_nc(reps=1):
    nc = bacc.Bacc("TRN2", target_bir_lowering=False, debug=False)

    x_h = nc.dram_tensor("x", [C, 64, 64], F32, kind="ExternalInput")
    y_h = nc.dram_tensor("y", [C, 64, 64], F32, kind="ExternalInput")
    wq_h = nc.dram_tensor("Wq", [CQK, C], F32, kind="ExternalInput")
    bq_h = nc.dram_tensor("bq", [CQK], F32, kind="ExternalInput")
    wk_h = nc.dram_tensor("Wk", [CQK, C], F32, kind="ExternalInput")
    bk_h = nc.dram_tensor("bk", [CQK], F32, kind="ExternalInput")
    wv_h = nc.dram_tensor("Wv", [C, C], F32, kind="ExternalInput")
    bv_h = nc.dram_tensor("bv", [C], F32, kind="ExternalInput")
    out_h = nc.dram_tensor("out", [C, 64, 64], F32, kind="ExternalOutput")

    x_v = x_h.rearrange("c h w -> c (h w)")
    y_v = y_h.rearrange("c h w -> c (h w)")
    out_v = out_h.rearrange("c h w -> c (h w)")

    def emit_once(tc, nc, rep):
      with ExitStack() as stk:
        consts = stk.enter_context(tc.tile_pool(name=f"consts{rep}", bufs=1))
        xy = stk.enter_context(tc.tile_pool(name=f"xy{rep}", bufs=1))
        big = stk.enter_context(tc.tile_pool(name=f"big{rep}", bufs=1))
        ppool = stk.enter_context(tc.tile_pool(name=f"ppool{rep}", bufs=3))
        npool = stk.enter_context(tc.tile_pool(name=f"npool{rep}", bufs=6))
        spool = stk.enter_context(tc.tile_pool(name=f"spool{rep}", bufs=4))

        ident = consts.tile([128, 128], F32, name="ident", tag="ident")
        make_identity(nc, ident)

        # ---- load inputs ----
        xin = []
        yin = []
        for cc in range(2):
            xt = xy.tile([128, HW], F32R, name=f"xin{cc}", tag=f"xin{cc}")
            yt = xy.tile([128, HW], F32R, name=f"yin{cc}", tag=f"yin{cc}")
            for q in range(4):
                qs = slice(q * (HW // 4), (q + 1) * (HW // 4))
                nc.sync.dma_start(
                    out=xt[:, qs],
                    in_=x_v[cc * 128:(cc + 1) * 128, qs].bitcast(F32R),
                )
                nc.sync.dma_start(
                    out=yt[:, qs],
                    in_=y_v[cc * 128:(cc + 1) * 128, qs].bitcast(F32R),
                )
            xin.append(xt)
            yin.append(yt)

        wq_sb = consts.tile([CQK, C], F32, name="wq_sb", tag="wq_sb")
        nc.sync.dma_start(out=wq_sb, in_=wq_h[:, :])
        wk_sb = consts.tile([CQK, C], F32, name="wk_sb", tag="wk_sb")
        nc.sync.dma_start(out=wk_sb, in_=wk_h[:, :])
        wv_sb = []
        for cc in range(2):
            t = consts.tile([128, C], F32, name=f"wv_sb{cc}", tag=f"wv_sb{cc}")
            nc.sync.dma_start(out=t, in_=wv_h[cc * 128:(cc + 1) * 128, :])
            wv_sb.append(t)

        # biases: bq/bk replicated 4x partition-wise -> [128, 1]
        bq_rep = consts.tile([128, 1], F32, name="bq_rep", tag="bq_rep")
        bk_rep = consts.tile([128, 1], F32, name="bk_rep", tag="bk_rep")
        for r in range(4):
            nc.sync.dma_start(
                out=bq_rep[32 * r:32 * (r + 1), :],
                in_=bq_h.rearrange("(o u) -> o u", u=1),
            )
            nc.sync.dma_start(
                out=bk_rep[32 * r:32 * (r + 1), :],
                in_=bk_h.rearrange("(o u) -> o u", u=1),
            )
        # bv_aug row: [bv | 1.0], used as the K=1 tail matmul of V_aug^T
        bv_aug = consts.tile([1, C + 2], F32R, name="bv_aug", tag="bv_aug")
        nc.sync.dma_start(
            out=bv_aug[0:1, 0:C], in_=bv_h.rearrange("(u c) -> u c", u=1).bitcast(F32R)
        )
        # f32r memsets are rejected by walrus; memset f32 scratch, then
        # DVE-copy (which rounds) into the f32r tiles.
        scr = consts.tile([128, 130], F32, name="scr", tag="scr")
        nc.vector.memset(scr[:, 0:2], 0.0)
        nc.vector.memset(scr[:, 2:130], 1.0)
        nc.vector.tensor_copy(out=bv_aug[0:1, C:C + 1], in_=scr[0:1, 2:3])
        nc.vector.tensor_copy(out=bv_aug[0:1, C + 1:C + 2], in_=scr[0:1, 0:1])
        ones_row = consts.tile([1, 128], F32R, name="ones_row", tag="ones_row")
        nc.vector.tensor_copy(out=ones_row, in_=scr[0:1, 2:130])

        # persistent big tensors
        qrep = big.tile([128, HW], F32R, name="qrep", tag="qrep")
        krep = big.tile([128, HW], F32R, name="krep", tag="krep")
        vaug = big.tile([128, MCH, C + 2], BF16 if OUT_BF16 else F32R, name="vaug", tag="vaug")

        # stationaries for projections
        wqT_rep = []
        wkT_rep = []
        wvT_aug = []
        for cc in range(2):
            wqT_rep.append(
                consts.tile([128, 128], F32R, name=f"wqT{cc}", tag=f"wqT{cc}")
            )
            wkT_rep.append(
                consts.tile([128, 128], F32R, name=f"wkT{cc}", tag=f"wkT{cc}")
            )
            t = consts.tile([128, C + 2], F32R, name=f"wvT{cc}", tag=f"wvT{cc}")
            nc.vector.tensor_copy(out=t[:, C:C + 2], in_=scr[:, 0:2])
            wvT_aug.append(t)

        # ---- prep-phase PSUM pools (closed before the main loop) ----
        with tc.tile_pool(name="psum_pp", bufs=4, space="PSUM") as psum_pp, \
             tc.tile_pool(name="psum_tp", bufs=2, space="PSUM") as psum_tp:
            # transpose Wq/Wk chunks -> replicated stationaries
            for cc in range(2):
                tq = psum_tp.tile([128, 128], F32, name=f"tq{cc}", tag="tp")
                nc.tensor.transpose(
                    tq[:, 0:CQK],
                    wq_sb[0:CQK, cc * 128:(cc + 1) * 128],
                    ident[0:CQK, 0:CQK],
                )
                for r in range(4):
                    nc.vector.tensor_copy(
                        out=wqT_rep[cc][:, 32 * r:32 * (r + 1)], in_=tq[:, 0:CQK]
                    )
                tk = psum_tp.tile([128, 128], F32, name=f"tk{cc}", tag="tp")
                nc.tensor.transpose(
                    tk[:, 0:CQK],
                    wk_sb[0:CQK, cc * 128:(cc + 1) * 128],
                    ident[0:CQK, 0:CQK],
                )
                for r in range(4):
                    nc.vector.tensor_copy(
                        out=wkT_rep[cc][:, 32 * r:32 * (r + 1)], in_=tk[:, 0:CQK]
                    )
            # transpose Wv -> wvT_aug ([c' part, c free], col 256 = 0)
            for ccp in range(2):
                for cc in range(2):
                    tv = psum_tp.tile([128, 128], F32, name=f"tv{ccp}{cc}", tag="tp")
                    nc.tensor.transpose(
                        tv,
                        wv_sb[cc][:, ccp * 128:(ccp + 1) * 128],
                        ident,
                    )
                    nc.vector.tensor_copy(
                        out=wvT_aug[ccp][:, cc * 128:(cc + 1) * 128], in_=tv
                    )

            # Q'_rep / K'_rep projections (out = 4 stacked replicas of [32, n])
            for nn in range(NW):
                ns = slice(nn * WIN, (nn + 1) * WIN)
                qp = psum_pp.tile([128, WIN], F32, name=f"qp{nn}", tag="pp")
                nc.tensor.matmul(
                    out=qp, lhsT=wqT_rep[0], rhs=xin[0][:, ns],
                    start=True, stop=False,
                )
                nc.tensor.matmul(
                    out=qp, lhsT=wqT_rep[1], rhs=xin[1][:, ns],
                    start=False, stop=True,
                )
                nc.vector.tensor_scalar_add(out=qrep[:, ns], in0=qp, scalar1=bq_rep)
                kp = psum_pp.tile([128, WIN], F32, name=f"kp{nn}", tag="pp")
                nc.tensor.matmul(
                    out=kp, lhsT=wkT_rep[0], rhs=yin[0][:, ns],
                    start=True, stop=False,
                )
                nc.tensor.matmul(
                    out=kp, lhsT=wkT_rep[1], rhs=yin[1][:, ns],
                    start=False, stop=True,
                )
                nc.vector.tensor_scalar_add(out=krep[:, ns], in0=kp, scalar1=bk_rep)

            # V_aug^T: per m-chunk [128, 257] = yf^T Wv^T + [bv | 1]
            for mc in range(MCH):
                ms = slice(mc * 128, (mc + 1) * 128)
                vp = psum_pp.tile([128, WIN], F32, name=f"vp{mc}", tag="pp")
                nc.tensor.matmul(
                    out=vp[:, 0:C + 2], lhsT=yin[0][:, ms], rhs=wvT_aug[0],
                    start=True, stop=False,
                )
                nc.tensor.matmul(
                    out=vp[:, 0:C + 2], lhsT=yin[1][:, ms], rhs=wvT_aug[1],
                    start=False, stop=False,
                )
                nc.tensor.matmul(
                    out=vp[:, 0:C + 2], lhsT=ones_row, rhs=bv_aug,
                    start=False, stop=True,
                )
                nc.vector.tensor_copy(out=vaug[:, mc, :], in_=vp[:, 0:C + 2])

        # ---- main loop ----
        with tc.tile_pool(name="psum_s", bufs=2, space="PSUM") as psum_s, \
             tc.tile_pool(name="psum_o", bufs=4, space="PSUM") as psum_o:

            NG = MCH // 2  # 16 groups of 2 m-chunks

            def emit_s_group(w, g):
                """2 concurrent row-tiled score matmuls: S^T[m-chunks 2g..2g+1,
                n-window w] into a 2-bank PSUM tile. Alternating groups use
                alternating PE row-tile pairs (partitions 0-63 / 64-127) so
                consecutive groups overlap in the array and the double-
                buffered PSUM slots keep ACT's exp stream un-serialized."""
                sp = psum_s.tile([128, 2 * WIN], F32, name=f"sp{w}_{g}", tag="s")
                p = g % 2
                for u in range(2):
                    i = 2 * p + u
                    mc = 2 * g + u
                    prt = slice(32 * i, 32 * (i + 1))
                    nc.tensor.matmul(
                        out=sp[:, u * WIN:(u + 1) * WIN],
                        lhsT=krep[prt, mc * 128:(mc + 1) * 128],
                        rhs=qrep[prt, w * WIN:(w + 1) * WIN],
                        start=True, stop=True,
                        tile_position=(32 * i, 0),
                    )
                return sp

            for w in range(NW):
                opsum = [
                    psum_o.tile([128, C + 2], F32, name=f"o{w}_{j}", tag="o")
                    for j in range(NCH)
                ]
                sp = emit_s_group(w, 0)
                for g in range(NG):
                    pt = ppool.tile([128, 2 * WIN], BF16 if OUT_BF16 else F32R, name=f"pt{w}_{g}", tag="pt")
                    nc.scalar.activation(out=pt, in_=sp, func=AF.Exp)
                    if g + 1 < NG:
                        sp = emit_s_group(w, g + 1)
                    for u in range(2):
                        mc = 2 * g + u
                        for j in range(NCH):
                            nc.tensor.matmul(
                                out=opsum[j][:, 0:C + 2],
                                lhsT=pt[:, u * WIN + j * 128:u * WIN + (j + 1) * 128],
                                rhs=vaug[:, mc, :],
                                start=(mc == 0), stop=(mc == MCH - 1),
                            )
                # normalize: out^T[n, c] * (1/denom[n]); denom is col 256
                nsbs = []
                for j in range(NCH):
                    rec = npool.tile([128, 1], F32, name=f"rec{w}_{j}", tag="rec")
                    nc.vector.reciprocal(out=rec, in_=opsum[j][:, C:C + 1])
                    nsb = npool.tile([128, C], F32, name=f"nsb{w}_{j}", tag="nsb")
                    nc.vector.tensor_scalar_mul(
                        out=nsb, in0=opsum[j][:, 0:C], scalar1=rec
                    )
                    nsbs.append(nsb)
                # transpose back to [c, n] and store
                ost = [
                    spool.tile([128, WIN], F32, name=f"ost{w}_{cc}", tag="ost")
                    for cc in range(2)
                ]
                for j in range(NCH):
                    for cc in range(2):
                        tp = psum_o.tile([128, C + 2], F32, name=f"tp{w}_{j}{cc}", tag="o")
                        nc.tensor.transpose(
                            tp[:, 0:128],
                            nsbs[j][:, cc * 128:(cc + 1) * 128],
                            ident,
                        )
                        nc.vector.tensor_copy(
                            out=ost[cc][:, j * 128:(j + 1) * 128], in_=tp[:, 0:128]
                        )
                for cc in range(2):
                    nc.sync.dma_start(
                        out=out_v[cc * 128:(cc + 1) * 128, w * WIN:(w + 1) * WIN],
                        in_=ost[cc],
                    )

    with tile.TileContext(nc) as tc:
        for rep in range(reps):
            emit_once(tc, nc, rep)

    nc.compile()
    return nc


def _get_nc():
    if "nc" not in _CACHE:
        _CACHE["nc"] = _build_nc()
    return _CACHE["nc"]


class _Runner:
    """One-time jitted SPMD executor for the bass program (mirrors
    bass2jax.run_bass_via_pjrt, but keeps the jitted callable for reuse)."""

    def __init__(self, nc):
        import jax
        import concourse.mybir as mybir_
        from concourse import bass2jax
        from jax.experimental.shard_map import shard_map
        from jax.sharding import Mesh, PartitionSpec

        bass2jax.install_neuronx_cc_hook()
        self.jax = jax
        self.nc = nc

        partition_name = (
            nc.partition_id_tensor.name if nc.partition_id_tensor else None
        )
        in_names, out_names, out_avals, zero_outs = [], [], [], []
        for alloc in nc.m.functions[0].allocations:
            if not isinstance(alloc, mybir_.MemoryLocationSet):
                continue
            name = alloc.memorylocations[0].name
            if alloc.kind == "ExternalInput":
                if name != partition_name:
                    in_names.append(name)
            elif alloc.kind == "ExternalOutput":
                out_names.append(name)
                shape = tuple(alloc.tensor_shape)
                dtype = mybir_.dt.np(alloc.dtype)
                out_avals.append(jax.core.ShapedArray(shape, dtype))
                zero_outs.append(np.zeros(shape, dtype))
        self.in_names = list(in_names)
        self.out_names = out_names
        self.zero_outs = zero_outs
        n_params = len(in_names)
        n_outs = len(out_avals)
        all_in_names = in_names + out_names
        if partition_name is not None:
            all_in_names = all_in_names + [partition_name]
        donate = tuple(range(n_params, n_params + n_outs))
        self.n_params = n_params

        def _body(*args):
            operands = list(args)
            if partition_name is not None:
                operands.append(bass2jax.partition_id_tensor())
            outs = bass2jax._bass_exec_p.bind(
                *operands,
                out_avals=tuple(out_avals),
                in_names=tuple(all_in_names),
                out_names=tuple(out_names),
                lowering_input_output_aliases=(),
                sim_require_finite=True,
                sim_require_nnan=True,
                nc=nc,
            )
            return tuple(outs)

        devices = jax.devices()[:N_CORES]
        self.mesh = Mesh(np.asarray(devices), ("core",))
        in_specs = (PartitionSpec("core"),) * (n_params + n_outs)
        out_specs = (PartitionSpec("core"),) * n_outs
        self.sharded = jax.jit(
            shard_map(
                _body, mesh=self.mesh, in_specs=in_specs, out_specs=out_specs,
                check_rep=False,
            ),
            donate_argnums=donate,
            keep_unused=True,
        )

    def make_zeros(self):
        return [
            np.zeros((N_CORES * z.shape[0], *z.shape[1:]), z.dtype)
            for z in self.zero_outs
        ]

    def concat_inputs(self, in_maps):
        return [
            np.concatenate([np.asarray(m[name]) for m in in_maps], axis=0)
            for name in self.in_names
        ]

    def run(self, concat_in, zeros):
        outs = self.sharded(*concat_in, *zeros)
        return outs


def _get_runner():
    if "runner" not in _CACHE:
        _CACHE["runner"] = _Runner(_get_nc())
    return _CACHE["runner"]


def kernel(x, y, Wq, bq, Wk, bk, Wv, bv):
    r = _get_runner()
    x = np.ascontiguousarray(np.asarray(x, dtype=np.float32))
    y = np.ascontiguousarray(np.asarray(y, dtype=np.float32))
    Wq = np.ascontiguousarray(np.asarray(Wq, dtype=np.float32))
    bq = np.ascontiguousarray(np.asarray(bq, dtype=np.float32))
    Wk = np.ascontiguousarray(np.asarray(Wk, dtype=np.float32))
    bk = np.ascontiguousarray(np.asarray(bk, dtype=np.float32))
    Wv = np.ascontiguousarray(np.asarray(Wv, dtype=np.float32))
    bv = np.ascontiguousarray(np.asarray(bv, dtype=np.float32))

    in_maps = [
        {
            "x": x[b], "y": y[b],
            "Wq": Wq, "bq": bq, "Wk": Wk, "bk": bk, "Wv": Wv, "bv": bv,
        }
        for b in range(B)
    ]
    concat_in = r.concat_inputs(in_maps)
    outs = r.run(concat_in, r.make_zeros())
    out = np.asarray(outs[0])  # [8*256, 64, 64]
    return out.reshape(B, C, 64, 64)


# revision 15
# speedup vs baseline: 321.3237x; 24.1813x over previous
"""Trainium2 Bass kernel for nn_CrossAttention (B=8, C=256, CQK=32, H=W=64).

Per-batch cross attention:
    Q = Wq @ xf        [32, 4096]   (+bq)
    K = Wk @ yf        [32, 4096]   (+bk)
    V = Wv @ yf        [256, 4096]  (+bv)
    S = Q^T K          [4096, 4096]
    P = softmax(S, axis=-1)
    out = V @ P^T      [256, 4096]

Sharding: pure data-parallel over batch — core b handles batch b. Weights
replicated. No collectives.

Per-core algorithm (all on-chip, S/P never touch HBM):
  * Q'_rep/K'_rep: projections with the [256,32] transposed weights
    replicated 4x along PE columns, so each 32-partition block of SBUF
    holds a full copy of Q'/K' — feeds 4x row-tiled (K=32) score matmuls.
  * S^T computed in [m, n] layout (m = key index on partitions) via
    4 concurrent row-tiled matmuls (tile_position=(32i,0)), one 512-col
    n-window x 4 m-chunks per group, into 4 PSUM banks.
  * One wide ACT exp over the 4 banks -> P^T tile [128, 2048] in SBUF.
  * out^T[n, c] accumulated in PSUM over all m: stationary = P^T block
    [128m, 128n], moving = V_aug^T[m-chunk] = [V^T | 1] (257 cols). The
    ones column makes PSUM col 256 the softmax denominator for free.
  * normalize by 1/denominator (per-partition broadcast), PE-transpose
    back to [c, n], DMA out.

Unsafe softmax (no max subtraction) is used: scores are ~N(0, 32) for
this problem size, |S| < ~40, exp stays well inside f32 range.

Matmuls use dtype float32r (full f32 storage, single-pass PE matmul —
full rate at free-dim >= 256).
"""

from contextlib import ExitStack

import numpy as np

import concourse.bass as bass
import concourse.mybir as mybir
import concourse.tile as tile
from concourse import bacc
from concourse.bass_utils import run_bass_kernel_spmd
from concourse.masks import make_identity

F32 = mybir.dt.float32
F32R = mybir.dt.float32r
BF16 = mybir.dt.bfloat16
AF = mybir.ActivationFunctionType

# dtype for the big out^T = P^T-stationary x V_aug matmuls (1024 of them).
# float32r: ~10-bit-mantissa single-pass fp32; bfloat16: guaranteed
# 1 cyc/row + fast weight load.
OUT_BF16 = False

B = 8
C = 256          # channels
CQK = 32         # q/k projection dim
HW = 4096        # 64*64 pixels
NW = 8           # n-windows
WIN = HW // NW   # 512 = n-window size
NCH = WIN // 128  # 4 = n-chunks (128) per window
MCH = HW // 128  # 32 m-chunks
MG = MCH // 4    # 8 m-groups of 4 chunks

N_CORES = 8

_CACHE = {}


def _r(ap):
    """View an f32 AP as float32r for full-rate single-pass PE matmul."""
    return ap.bitcast(F32R)


def _build_nc(reps=1):
    nc = bacc.Bacc("TRN2", target_bir_lowering=False, debug=False)

    x_h = nc.dram_tensor("x", [C, 64, 64], F32, kind="ExternalInput")
    y_h = nc.dram_tensor("y", [C, 64, 64], F32, kind="ExternalInput")
    wq_h = nc.dram_tensor("Wq", [CQK, C], F32, kind="ExternalInput")
    bq_h = nc.dram_tensor("bq", [CQK], F32, kind="ExternalInput")
    wk_h = nc.dram_tensor("Wk", [CQK, C], F32, kind="ExternalInput")
    bk_h = nc.dram_tensor("bk", [CQK], F32, kind="ExternalInput")
    wv_h = nc.dram_tensor("Wv", [C, C], F32, kind="ExternalInput")
    bv_h = nc.dram_tensor("bv", [C], F32, kind="ExternalInput")
    out_h = nc.dram_tensor("out", [C, 64, 64], F32, kind="ExternalOutput")

    x_v = x_h.rearrange("c h w -> c (h w)")
    y_v = y_h.rearrange("c h w -> c (h w)")
    out_v = out_h.rearrange("c h w -> c (h w)")

    def emit_once(tc, nc, rep):
      with ExitStack() as stk:
        consts = stk.enter_context(tc.tile_pool(name=f"consts{rep}", bufs=1))
        xy = stk.enter_context(tc.tile_pool(name=f"xy{rep}", bufs=1))
        big = stk.enter_context(tc.tile_pool(name=f"big{rep}", bufs=1))
        ppool = stk.enter_context(tc.tile_pool(name=f"ppool{rep}", bufs=3))
        npool = stk.enter_context(tc.tile_pool(name=f"npool{rep}", bufs=6))
        spool = stk.enter_context(tc.tile_pool(name=f"spool{rep}", bufs=4))

        ident = consts.tile([128, 128], F32, name="ident", tag="ident")
        make_identity(nc, ident)

        # ---- load inputs ----
        xin = []
        yin = []
        for cc in range(2):
            xt = xy.tile([128, HW], F32R, name=f"xin{cc}", tag=f"xin{cc}")
            yt = xy.tile([128, HW], F32R, name=f"yin{cc}", tag=f"yin{cc}")
            for q in range(4):
                qs = slice(q * (HW // 4), (q + 1) * (HW // 4))
                nc.sync.dma_start(
                    out=xt[:, qs],
                    in_=x_v[cc * 128:(cc + 1) * 128, qs].bitcast(F32R),
                )
                nc.sync.dma_start(
                    out=yt[:, qs],
                    in_=y_v[cc * 128:(cc + 1) * 128, qs].bitcast(F32R),
                )
            xin.append(xt)
            yin.append(yt)

        wq_sb = consts.tile([CQK, C], F32, name="wq_sb", tag="wq_sb")
        nc.sync.dma_start(out=wq_sb, in_=wq_h[:, :])
        wk_sb = consts.tile([CQK, C], F32, name="wk_sb", tag="wk_sb")
        nc.sync.dma_start(out=wk_sb, in_=wk_h[:, :])
        wv_sb = []
        for cc in range(2):
            t = consts.tile([128, C], F32, name=f"wv_sb{cc}", tag=f"wv_sb{cc}")
            nc.sync.dma_start(out=t, in_=wv_h[cc * 128:(cc + 1) * 128, :])
            wv_sb.append(t)

        # biases: bq/bk replicated 4x partition-wise -> [128, 1]
        bq_rep = consts.tile([128, 1], F32, name="bq_rep", tag="bq_rep")
        bk_rep = consts.tile([128, 1], F32, name="bk_rep", tag="bk_rep")
        for r in range(4):
            nc.sync.dma_start(
                out=bq_rep[32 * r:32 * (r + 1), :],
                in_=bq_h.rearrange("(o u) -> o u", u=1),
            )
            nc.sync.dma_start(
                out=bk_rep[32 * r:32 * (r + 1), :],
                in_=bk_h.rearrange("(o u) -> o u", u=1),
            )
        # bv_aug row: [bv | 1.0], used as the K=1 tail matmul of V_aug^T
        bv_aug = consts.tile([1, C + 2], F32R, name="bv_aug", tag="bv_aug")
        nc.sync.dma_start(
            out=bv_aug[0:1, 0:C], in_=bv_h.rearrange("(u c) -> u c", u=1).bitcast(F32R)
        )
        # f32r memsets are rejected by walrus; memset f32 scratch, then
        # DVE-copy (which rounds) into the f32r tiles.
        scr = consts.tile([128, 130], F32, name="scr", tag="scr")
        nc.vector.memset(scr[:, 0:2], 0.0)
        nc.vector.memset(scr[:, 2:130], 1.0)
        nc.vector.tensor_copy(out=bv_aug[0:1, C:C + 1], in_=scr[0:1, 2:3])
        nc.vector.tensor_copy(out=bv_aug[0:1, C + 1:C + 2], in_=scr[0:1, 0:1])
        ones_row = consts.tile([1, 128], F32R, name="ones_row", tag="ones_row")
        nc.vector.tensor_copy(out=ones_row, in_=scr[0:1, 2:130])

        # persistent big tensors
        qrep = big.tile([128, HW], F32R, name="qrep", tag="qrep")
        krep = big.tile([128, HW], F32R, name="krep", tag="krep")
        vaug = big.tile([128, MCH, C + 2], BF16 if OUT_BF16 else F32R, name="vaug", tag="vaug")

        # stationaries for projections
        wqT_rep = []
        wkT_rep = []
        wvT_aug = []
        for cc in range(2):
            wqT_rep.append(
                consts.tile([128, 128], F32R, name=f"wqT{cc}", tag=f"wqT{cc}")
            )
            wkT_rep.append(
                consts.tile([128, 128], F32R, name=f"wkT{cc}", tag=f"wkT{cc}")
            )
            t = consts.tile([128, C + 2], F32R, name=f"wvT{cc}", tag=f"wvT{cc}")
            nc.vector.tensor_copy(out=t[:, C:C + 2], in_=scr[:, 0:2])
            wvT_aug.append(t)

        # ---- prep-phase PSUM pools (closed before the main loop) ----
        with tc.tile_pool(name="psum_pp", bufs=4, space="PSUM") as psum_pp, \
             tc.tile_pool(name="psum_tp", bufs=2, space="PSUM") as psum_tp:
            # transpose Wq/Wk chunks -> replicated stationaries
            for cc in range(2):
                tq = psum_tp.tile([128, 128], F32, name=f"tq{cc}", tag="tp")
                nc.tensor.transpose(
                    tq[:, 0:CQK],
                    wq_sb[0:CQK, cc * 128:(cc + 1) * 128],
                    ident[0:CQK, 0:CQK],
                )
                for r in range(4):
                    nc.vector.tensor_copy(
                        out=wqT_rep[cc][:, 32 * r:32 * (r + 1)], in_=tq[:, 0:CQK]
                    )
                tk = psum_tp.tile([128, 128], F32, name=f"tk{cc}", tag="tp")
                nc.tensor.transpose(
                    tk[:, 0:CQK],
                    wk_sb[0:CQK, cc * 128:(cc + 1) * 128],
                    ident[0:CQK, 0:CQK],
                )
                for r in range(4):
                    nc.vector.tensor_copy(
                        out=wkT_rep[cc][:, 32 * r:32 * (r + 1)], in_=tk[:, 0:CQK]
                    )
            # transpose Wv -> wvT_aug ([c' part, c free], col 256 = 0)
            for ccp in range(2):
                for cc in range(2):
                    tv = psum_tp.tile([128, 128], F32, name=f"tv{ccp}{cc}", tag="tp")
                    nc.tensor.transpose(
                        tv,
                        wv_sb[cc][:, ccp * 128:(ccp + 1) * 128],
                        ident,
                    )
                    nc.vector.tensor_copy(
                        out=wvT_aug[ccp][:, cc * 128:(cc + 1) * 128], in_=tv
                    )

            # Q'_rep / K'_rep projections (out = 4 stacked replicas of [32, n])
            for nn in range(NW):
                ns = slice(nn * WIN, (nn + 1) * WIN)
                qp = psum_pp.tile([128, WIN], F32, name=f"qp{nn}", tag="pp")
                nc.tensor.matmul(
                    out=qp, lhsT=wqT_rep[0], rhs=xin[0][:, ns],
                    start=True, stop=False,
                )
                nc.tensor.matmul(
                    out=qp, lhsT=wqT_rep[1], rhs=xin[1][:, ns],
                    start=False, stop=True,
                )
                nc.vector.tensor_scalar_add(out=qrep[:, ns], in0=qp, scalar1=bq_rep)
                kp = psum_pp.tile([128, WIN], F32, name=f"kp{nn}", tag="pp")
                nc.tensor.matmul(
                    out=kp, lhsT=wkT_rep[0], rhs=yin[0][:, ns],
                    start=True, stop=False,
                )
                nc.tensor.matmul(
                    out=kp, lhsT=wkT_rep[1], rhs=yin[1][:, ns],
                    start=False, stop=True,
                )
                nc.vector.tensor_scalar_add(out=krep[:, ns], in0=kp, scalar1=bk_rep)

            # V_aug^T: per m-chunk [128, 257] = yf^T Wv^T + [bv | 1]
            for mc in range(MCH):
                ms = slice(mc * 128, (mc + 1) * 128)
                vp = psum_pp.tile([128, WIN], F32, name=f"vp{mc}", tag="pp")
                nc.tensor.matmul(
                    out=vp[:, 0:C + 2], lhsT=yin[0][:, ms], rhs=wvT_aug[0],
                    start=True, stop=False,
                )
                nc.tensor.matmul(
                    out=vp[:, 0:C + 2], lhsT=yin[1][:, ms], rhs=wvT_aug[1],
                    start=False, stop=False,
                )
                nc.tensor.matmul(
                    out=vp[:, 0:C + 2], lhsT=ones_row, rhs=bv_aug,
                    start=False, stop=True,
                )
                nc.vector.tensor_copy(out=vaug[:, mc, :], in_=vp[:, 0:C + 2])

        # ---- main loop ----
        with tc.tile_pool(name="psum_s", bufs=2, space="PSUM") as psum_s, \
             tc.tile_pool(name="psum_o", bufs=4, space="PSUM") as psum_o:

            NG = MCH // 2  # 16 groups of 2 m-chunks

            def emit_s_group(w, g):
                """2 concurrent row-tiled score matmuls: S^T[m-chunks 2g..2g+1,
                n-window w] into a 2-bank PSUM tile. Alternating groups use
                alternating PE row-tile pairs (partitions 0-63 / 64-127) so
                consecutive groups overlap in the array and the double-
                buffered PSUM slots keep ACT's exp stream un-serialized."""
                sp = psum_s.tile([128, 2 * WIN], F32, name=f"sp{w}_{g}", tag="s")
                p = g % 2
                for u in range(2):
                    i = 2 * p + u
                    mc = 2 * g + u
                    prt = slice(32 * i, 32 * (i + 1))
                    nc.tensor.matmul(
                        out=sp[:, u * WIN:(u + 1) * WIN],
                        lhsT=krep[prt, mc * 128:(mc + 1) * 128],
                        rhs=qrep[prt, w * WIN:(w + 1) * WIN],
                        start=True, stop=True,
                        tile_position=(32 * i, 0),
                    )
                return sp

            for w in range(NW):
                opsum = [
                    psum_o.tile([128, C + 2], F32, name=f"o{w}_{j}", tag="o")
                    for j in range(NCH)
                ]
                sp = emit_s_group(w, 0)
                for g in range(NG):
                    pt = ppool.tile([128, 2 * WIN], BF16 if OUT_BF16 else F32R, name=f"pt{w}_{g}", tag="pt")
                    nc.scalar.activation(out=pt, in_=sp, func=AF.Exp)
                    if g + 1 < NG:
                        sp = emit_s_group(w, g + 1)
                    for u in range(2):
                        mc = 2 * g + u
                        for j in range(NCH):
                            nc.tensor.matmul(
                                out=opsum[j][:, 0:C + 2],
                                lhsT=pt[:, u * WIN + j * 128:u * WIN + (j + 1) * 128],
                                rhs=vaug[:, mc, :],
                                start=(mc == 0), stop=(mc == MCH - 1),
                            )
                # normalize: out^T[n, c] * (1/denom[n]); denom is col 256
                nsbs = []
                for j in range(NCH):
                    rec = npool.tile([128, 1], F32, name=f"rec{w}_{j}", tag="rec")
                    nc.vector.reciprocal(out=rec, in_=opsum[j][:, C:C + 1])
                    nsb = npool.tile([128, C], F32, name=f"nsb{w}_{j}", tag="nsb")
                    nc.vector.tensor_scalar_mul(
                        out=nsb, in0=opsum[j][:, 0:C], scalar1=rec
                    )
                    nsbs.append(nsb)
                # transpose back to [c, n] and store
                ost = [
                    spool.tile([128, WIN], F32, name=f"ost{w}_{cc}", tag="ost")
                    for cc in range(2)
                ]
                for j in range(NCH):
                    for cc in range(2):
                        tp = psum_o.tile([128, C + 2], F32, name=f"tp{w}_{j}{cc}", tag="o")
                        nc.tensor.transpose(
                            tp[:, 0:128],
                            nsbs[j][:, cc * 128:(cc + 1) * 128],
                            ident,
                        )
                        nc.vector.tensor_copy(
                            out=ost[cc][:, j * 128:(j + 1) * 128], in_=tp[:, 0:128]
                        )
                for cc in range(2):
                    nc.sync.dma_start(
                        out=out_v[cc * 128:(cc + 1) * 128, w * WIN:(w + 1) * WIN],
                        in_=ost[cc],
                    )

    with tile.TileContext(nc) as tc:
        for rep in range(reps):
            emit_once(tc, nc, rep)

    nc.compile()
    return nc


def _get_nc():
    if "nc" not in _CACHE:
        _CACHE["nc"] = _build_nc()
    return _CACHE["nc"]


class _Runner:
    """One-time jitted SPMD executor for the bass program (mirrors
    bass2jax.run_bass_via_pjrt, but keeps the jitted callable for reuse)."""

    def __init__(self, nc):
        import jax
        import concourse.mybir as mybir_
        from concourse import bass2jax
        from jax.experimental.shard_map import shard_map
        from jax.sharding import Mesh, PartitionSpec

        bass2jax.install_neuronx_cc_hook()
        self.jax = jax
        self.nc = nc

        partition_name = (
            nc.partition_id_tensor.name if nc.partition_id_tensor else None
        )
        in_names, out_names, out_avals, zero_outs = [], [], [], []
        for alloc in nc.m.functions[0].allocations:
            if not isinstance(alloc, mybir_.MemoryLocationSet):
                continue
            name = alloc.memorylocations[0].name
            if alloc.kind == "ExternalInput":
                if name != partition_name:
                    in_names.append(name)
            elif alloc.kind == "ExternalOutput":
                out_names.append(name)
                shape = tuple(alloc.tensor_shape)
                dtype = mybir_.dt.np(alloc.dtype)
                out_avals.append(jax.core.ShapedArray(shape, dtype))
                zero_outs.append(np.zeros(shape, dtype))
        self.in_names = list(in_names)
        self.out_names = out_names
        self.zero_outs = zero_outs
        n_params = len(in_names)
        n_outs = len(out_avals)
        all_in_names = in_names + out_names
        if partition_name is not None:
            all_in_names = all_in_names + [partition_name]
        donate = tuple(range(n_params, n_params + n_outs))
        self.n_params = n_params

        def _body(*args):
            operands = list(args)
            if partition_name is not None:
                operands.append(bass2jax.partition_id_tensor())
            outs = bass2jax._bass_exec_p.bind(
                *operands,
                out_avals=tuple(out_avals),
                in_names=tuple(all_in_names),
                out_names=tuple(out_names),
                lowering_input_output_aliases=(),
                sim_require_finite=True,
                sim_require_nnan=True,
                nc=nc,
            )
            return tuple(outs)

        devices = jax.devices()[:N_CORES]
        self.mesh = Mesh(np.asarray(devices), ("core",))
        in_specs = (PartitionSpec("core"),) * (n_params + n_outs)
        out_specs = (PartitionSpec("core"),) * n_outs
        self.sharded = jax.jit(
            shard_map(
                _body, mesh=self.mesh, in_specs=in_specs, out_specs=out_specs,
                check_rep=False,
            ),
            donate_argnums=donate,
            keep_unused=True,
        )

    def make_zeros(self):
        return [
            np.zeros((N_CORES * z.shape[0], *z.shape[1:]), z.dtype)
            for z in self.zero_outs
        ]

    def concat_inputs(self, in_maps):
        return [
            np.concatenate([np.asarray(m[name]) for m in in_maps], axis=0)
            for name in self.in_names
        ]

    def run(self, concat_in, zeros):
        outs = self.sharded(*concat_in, *zeros)
        return outs


def _get_runner():
    if "runner" not in _CACHE:
        _CACHE["runner"] = _Runner(_get_nc())
    return _CACHE["runner"]


def kernel(x, y, Wq, bq, Wk, bk, Wv, bv):
    r = _get_runner()
    x = np.ascontiguousarray(np.asarray(x, dtype=np.float32))
    y = np.ascontiguousarray(np.asarray(y, dtype=np.float32))
    Wq = np.ascontiguousarray(np.asarray(Wq, dtype=np.float32))
    bq = np.ascontiguousarray(np.asarray(bq, dtype=np.float32))
    Wk = np.ascontiguousarray(np.asarray(Wk, dtype=np.float32))
    bk = np.ascontiguousarray(np.asarray(bk, dtype=np.float32))
    Wv = np.ascontiguousarray(np.asarray(Wv, dtype=np.float32))
    bv = np.ascontiguousarray(np.asarray(bv, dtype=np.float32))

    in_maps = [
        {
            "x": x[b], "y": y[b],
            "Wq": Wq, "bq": bq, "Wk": Wk, "bk": bk, "Wv": Wv, "bv": bv,
        }
        for b in range(B)
    ]
    concat_in = r.concat_inputs(in_maps)
    outs = r.run(concat_in, r.make_zeros())
    out = np.asarray(outs[0])  # [8*256, 64, 64]
    return out.reshape(B, C, 64, 64)
